# revision 5
# baseline (speedup 1.0000x reference)
"""Trainium2 Bass kernel for CIN (Compressed Interaction Network).

Problem: B=1024, F0=32, D=32, HID=[128,128,128], linear activations.
  layer k: z_k[b,d,(f,g)] = x0[b,f,d] * s_k[b,g,d];  h_k = z_k @ W_k + b_k
  s_{k+1} = h_k (transposed view);  out = concat_k sum_d h_k  -> (B, 384)

Strategy (8 cores, batch-sharded 128 samples/core):
  "Orientation B" layout: features on partitions, n=(b,d) on free dim.
    xT[f, n] = x0[b,f,d];  h_kT[h, n] (matmul output layout == next layer's
    state layout, so no transposes in the main chain).
  z tiles (128 part = one f (L1) or 4f x 32g (L0), 512 free) are built by DVE
  tensor-multiply of the state tile with a broadcast tile that the PE
  produces via ones/one-hot stationary matmuls (float32r, 1 cyc/row).
  Main GEMMs in float32r (~1.5e-4 rel err, 4x faster than fp32).
  Layer 2's full map is never materialized: out2 = vec(x0 @ h1^T) @ W2 + D*b2
  via per-sample bilinear contractions using a block-diagonal masked moving
  operand (one 128x128 fp32 matmul per 4-sample tile).
"""
import sys

sys.path.insert(0, "/opt/trn_rl_repo")

import numpy as np

import concourse.bass as bass
import concourse.tile as tile
from concourse import bacc, mybir
from concourse.bass_utils import run_bass_kernel_spmd

NCORES = 8
B, F0, D = 1024, 32, 32
H = 128
BL = B // NCORES          # samples per core
NTOT = BL * D             # 4096 n-columns per core
NJ = 512                  # n-chunk (one PSUM bank)
NCHUNK = NTOT // NJ       # 8
SPJ = NJ // D             # samples per n-chunk (16)
TS = 4                    # samples per 128-row tile in L2 (128 = 4*32)
NT = BL // TS             # 32 L2 tiles

f32 = mybir.dt.float32
f32r = mybir.dt.float32r

_cache = {}


def _build_program():
    nc = bacc.Bacc("TRN2", target_bir_lowering=False, debug=False,
                   num_devices=NCORES)

    # ---- DRAM I/O ----
    xT_d = nc.dram_tensor("xT", [F0, NTOT], f32, kind="ExternalInput").ap()
    W0_d = nc.dram_tensor("W0", [F0 * F0, H], f32, kind="ExternalInput").ap()
    W1_d = nc.dram_tensor("W1", [F0 * H, H], f32, kind="ExternalInput").ap()
    W2_d = nc.dram_tensor("W2", [F0 * H, H], f32, kind="ExternalInput").ap()
    b0_d = nc.dram_tensor("b0c", [H, 1], f32, kind="ExternalInput").ap()
    b1_d = nc.dram_tensor("b1c", [H, 1], f32, kind="ExternalInput").ap()
    b2_d = nc.dram_tensor("b2c", [H, 1], f32, kind="ExternalInput").ap()
    S_d = nc.dram_tensor("S", [F0, 8, 128], f32r, kind="ExternalInput").ap()
    E_d = nc.dram_tensor("E", [F0, F0, 128], f32r, kind="ExternalInput").ap()
    BLK_d = nc.dram_tensor("BLK", [128, TS], f32, kind="ExternalInput").ap()
    id_d = nc.dram_tensor("ident", [128, 128], f32, kind="ExternalInput").ap()
    out_d = nc.dram_tensor("out", [BL, 3 * H], f32, kind="ExternalOutput").ap()

    with tile.TileContext(nc) as tc:
        with tc.tile_pool(name="const", bufs=1) as cpool, \
             tc.tile_pool(name="zpool", bufs=4) as zpool, \
             tc.tile_pool(name="stg", bufs=1) as stg, \
             tc.tile_pool(name="h1a", bufs=2) as h1apool, \
             tc.tile_pool(name="xd", bufs=2) as xdpool, \
             tc.tile_pool(name="bcast_ps", bufs=3, space="PSUM") as bps, \
             tc.tile_pool(name="acc_ps", bufs=1, space="PSUM") as aps, \
             tc.tile_pool(name="l2_ps", bufs=3, space="PSUM") as l2ps:

            # ---- load constants / weights ----
            xT = cpool.tile([F0, NTOT], f32)
            nc.sync.dma_start(xT[:], xT_d)
            xTr = cpool.tile([F0, NTOT], f32r)
            nc.vector.tensor_copy(xTr[:], xT[:])

            W0s = stg.tile([128, F0, H], f32, tag="wstg")
            nc.sync.dma_start(W0s[:, :F0 * F0 // 128, :],
                              W0_d.rearrange("(c p) h -> p c h", p=128))
            W0r = cpool.tile([128, F0 * F0 // 128, H], f32r)
            nc.vector.tensor_copy(W0r[:], W0s[:, :F0 * F0 // 128, :])

            W1s = stg.tile([128, F0, H], f32, tag="wstg")
            nc.sync.dma_start(W1s[:], W1_d.rearrange("(c p) h -> p c h", p=128))
            W1r = cpool.tile([128, F0, H], f32r)
            nc.vector.tensor_copy(W1r[:], W1s[:])

            # W2 stays fp32 (L2 is tiny; avoids f32r pairing constraints)
            W2s = cpool.tile([128, F0, H], f32)
            nc.sync.dma_start(W2s[:], W2_d.rearrange("(c p) h -> p c h", p=128))

            b0c = cpool.tile([H, 1], f32)
            b1c = cpool.tile([H, 1], f32)
            b2c = cpool.tile([H, 1], f32)
            nc.sync.dma_start(b0c[:], b0_d)
            nc.sync.dma_start(b1c[:], b1_d)
            nc.sync.dma_start(b2c[:], b2_d)
            b2x = cpool.tile([H, 1], f32)
            nc.vector.tensor_scalar_mul(b2x[:], b2c[:], float(D))

            Sr = cpool.tile([F0, 8, 128], f32r)
            nc.sync.dma_start(Sr[:], S_d)

            BLKs = cpool.tile([128, TS], f32)
            nc.sync.dma_start(BLKs[:], BLK_d)
            ident = cpool.tile([128, 128], f32)
            nc.sync.dma_start(ident[:], id_d)

            Er = cpool.tile([F0, F0, 128], f32r)
            nc.sync.dma_start(Er[:], E_d)

            # XG[p, n] = xT[p % 32, n]
            XG = cpool.tile([128, NTOT], f32)
            for q in range(4):
                nc.vector.tensor_copy(XG[q * F0:(q + 1) * F0, :], xT[:])

            # ---- state and output tensors ----
            h0T = cpool.tile([H, NTOT], f32)
            h1T = cpool.tile([H, NTOT], f32)
            out0T = cpool.tile([H, BL], f32)
            out1T = cpool.tile([H, BL], f32)
            out2s = cpool.tile([H, BL], f32)
            XA = cpool.tile([128, NT, F0], f32)   # [(s,d), t, f]
            Pn = cpool.tile([128, NT * 128], f32)  # [g, (t, f, s)]
            out_all = cpool.tile([BL, 3 * H], f32)

            # ================= main layers, per n-chunk =================
            for j in range(NCHUNK):
                nsl = bass.ts(j, NJ)

                # ----- layer 0 -----
                h0ps = aps.tile([128, NJ], f32, tag="h0acc")
                for c in range(8):
                    x4 = bps.tile([128, NJ], f32, tag="bc")
                    nc.tensor.matmul(x4[:], Sr[:, c, :], xTr[:, nsl],
                                     start=True, stop=True)
                    z0 = zpool.tile([128, NJ], f32r, tag="z")
                    nc.vector.tensor_mul(z0[:], XG[:, nsl], x4[:])
                    nc.tensor.matmul(h0ps[:], W0r[:, c, :], z0[:],
                                     start=(c == 0), stop=(c == 7))
                nc.scalar.activation(h0T[:, nsl], h0ps[:],
                                     mybir.ActivationFunctionType.Identity,
                                     bias=b0c[:])

                # ----- layer 1 -----
                h1ps = aps.tile([128, NJ], f32, tag="h1acc")
                for f in range(F0):
                    xb = bps.tile([128, NJ], f32, tag="bc")
                    nc.tensor.matmul(xb[:], Er[:, f, :], xTr[:, nsl],
                                     start=True, stop=True)
                    z1 = zpool.tile([128, NJ], f32r, tag="z")
                    nc.vector.tensor_mul(z1[:], h0T[:, nsl], xb[:])
                    nc.tensor.matmul(h1ps[:], W1r[:, f, :], z1[:],
                                     start=(f == 0), stop=(f == F0 - 1))
                nc.scalar.activation(h1T[:, nsl], h1ps[:],
                                     mybir.ActivationFunctionType.Identity,
                                     bias=b1c[:])

                # ----- d-reductions for out0 / out1 -----
                bsl = bass.ts(j, SPJ)
                nc.vector.reduce_sum(
                    out0T[:, bsl],
                    h0T[:, nsl].rearrange("p (b d) -> p b d", d=D),
                    axis=mybir.AxisListType.X)
                nc.vector.reduce_sum(
                    out1T[:, bsl],
                    h1T[:, nsl].rearrange("p (b d) -> p b d", d=D),
                    axis=mybir.AxisListType.X)

            # ================= layer 2 (reduced bilinear form) ==========
            for t in range(NT):
                tsl = bass.ts(t, 128)
                # XA_t = (xT[:, t*128:+128])^T  -> (128 rows=(s,d), 32 f)
                xa_ps = l2ps.tile([128, 128], f32, tag="l2")
                nc.tensor.transpose(xa_ps[:, :F0], xT[:, tsl], ident[:F0, :F0])
                nc.scalar.activation(XA[:, t, :], xa_ps[:, :F0],
                                     mybir.ActivationFunctionType.Copy)
                # H1A_t = (h1T[:, t*128:+128])^T -> (128 rows=(s,d), 128 g)
                h1a_ps = l2ps.tile([128, 128], f32, tag="l2")
                nc.tensor.transpose(h1a_ps[:], h1T[:, tsl], ident[:])
                h1a = h1apool.tile([128, 128], f32)
                nc.scalar.activation(h1a[:], h1a_ps[:],
                                     mybir.ActivationFunctionType.Copy)
                # XD_t[(s,d), (f,s')] = XA_t[(s,d), f] * (s == s')
                xd = xdpool.tile([128, F0, TS], f32)
                nc.vector.tensor_mul(
                    xd[:],
                    XA[:, t, :, None].to_broadcast((128, F0, TS)),
                    BLKs[:, None, :].to_broadcast((128, F0, TS)))
                # Pn_t[g, (f, s)] = sum_d h1[b_s, g, d] * x0[b_s, f, d]
                pn_ps = l2ps.tile([128, 128], f32, tag="l2")
                nc.tensor.matmul(pn_ps[:], h1a[:],
                                 xd[:].rearrange("p f s -> p (f s)"),
                                 start=True, stop=True)
                nc.scalar.activation(Pn[:, tsl], pn_ps[:],
                                     mybir.ActivationFunctionType.Copy)

            # out2T[h, b] = sum_f W2[(f,:),h]^T @ Pn[:, (:, f, :)]
            out2ps = l2ps.tile([128, BL], f32, tag="l2")
            PnV = Pn[:].rearrange("p (t f s) -> p t f s", f=F0, s=TS)
            for f in range(F0):
                nc.tensor.matmul(out2ps[:], W2s[:, f, :], PnV[:, :, f, :],
                                 start=(f == 0), stop=(f == F0 - 1))
            nc.vector.tensor_scalar_add(out2s[:], out2ps[:], b2x[:])

            # ============ transpose outputs to (b, h) and store =========
            for k, src in enumerate((out0T, out1T, out2s)):
                ops_ = l2ps.tile([128, 128], f32, tag="l2")
                nc.tensor.transpose(ops_[:], src[:], ident[:])
                nc.scalar.activation(out_all[:, bass.ts(k, H)], ops_[:],
                                     mybir.ActivationFunctionType.Copy)
            nc.sync.dma_start(out_d, out_all[:])

    nc.compile()
    return nc


def _consts():
    S = np.zeros((F0, 8, 128), np.float32)
    for c in range(8):
        for m in range(128):
            S[4 * c + m // 32, c, m] = 1.0
    E = np.zeros((F0, F0, 128), np.float32)
    for f in range(F0):
        E[f, f, :] = 1.0
    BLK = np.zeros((128, TS), np.float32)
    for p in range(128):
        BLK[p, p // 32] = 1.0
    ident = np.eye(128, dtype=np.float32)
    return S, E, BLK, ident


def kernel(inputs, W0, W1, W2, b0, b1, b2, field_size, embedding_size):
    x0 = np.ascontiguousarray(np.asarray(inputs, np.float32).reshape(B, F0, D))
    W0 = np.ascontiguousarray(np.asarray(W0, np.float32))
    W1 = np.ascontiguousarray(np.asarray(W1, np.float32))
    W2 = np.ascontiguousarray(np.asarray(W2, np.float32))
    b0 = np.asarray(b0, np.float32).reshape(H, 1)
    b1 = np.asarray(b1, np.float32).reshape(H, 1)
    b2 = np.asarray(b2, np.float32).reshape(H, 1)

    if "nc" not in _cache:
        _cache["nc"] = _build_program()
    nc = _cache["nc"]

    S, E, BLK, ident = _consts()
    in_maps = []
    for c in range(NCORES):
        xs = x0[c * BL:(c + 1) * BL]                      # (128, 32, 32)
        xT = np.ascontiguousarray(xs.transpose(1, 0, 2).reshape(F0, NTOT))
        in_maps.append({
            "xT": xT, "W0": W0, "W1": W1, "W2": W2,
            "b0c": b0, "b1c": b1, "b2c": b2,
            "S": S.copy(), "E": E.copy(), "BLK": BLK.copy(), "ident": ident.copy(),
        })

    res = run_bass_kernel_spmd(nc, in_maps, list(range(NCORES)),
                               **_cache.get("run_kwargs", {}))
    _cache["last_result"] = res
    out = np.concatenate([res.results[c]["out"] for c in range(NCORES)], axis=0)
    return out.astype(np.float32)


# revision 6
# speedup vs baseline: 1.0001x; 1.0001x over previous
"""Trainium2 Bass kernel for CIN (Compressed Interaction Network).

Problem: B=1024, F0=32, D=32, HID=[128,128,128], linear activations.
  layer k: z_k[b,d,(f,g)] = x0[b,f,d] * s_k[b,g,d];  h_k = z_k @ W_k + b_k
  s_{k+1} = h_k (transposed view);  out = concat_k sum_d h_k  -> (B, 384)

Strategy (8 cores, batch-sharded 128 samples/core):
  "Orientation B" layout: features on partitions, n=(b,d) on free dim.
    xT[f, n] = x0[b,f,d];  h_kT[h, n] (matmul output layout == next layer's
    state layout, so no transposes in the main chain).
  z tiles (128 part = one f (L1) or 4f x 32g (L0), 512 free) are built by DVE
  tensor-multiply of the state tile with a broadcast tile that the PE
  produces via ones/one-hot stationary matmuls (float32r, 1 cyc/row).
  Main GEMMs in float32r (~1.5e-4 rel err, 4x faster than fp32).
  Layer 2's full map is never materialized: out2 = vec(x0 @ h1^T) @ W2 + D*b2
  via per-sample bilinear contractions using a block-diagonal masked moving
  operand (one 128x128 fp32 matmul per 4-sample tile).
"""
import sys

sys.path.insert(0, "/opt/trn_rl_repo")

import numpy as np

import concourse.bass as bass
import concourse.tile as tile
from concourse import bacc, mybir
from concourse.bass_utils import run_bass_kernel_spmd

NCORES = 8
B, F0, D = 1024, 32, 32
H = 128
BL = B // NCORES          # samples per core
NTOT = BL * D             # 4096 n-columns per core
NJ = 512                  # n-chunk (one PSUM bank)
NCHUNK = NTOT // NJ       # 8
SPJ = NJ // D             # samples per n-chunk (16)
TS = 4                    # samples per 128-row tile in L2 (128 = 4*32)
NT = BL // TS             # 32 L2 tiles

f32 = mybir.dt.float32
f32r = mybir.dt.float32r

_cache = {}


def _build_program():
    nc = bacc.Bacc("TRN2", target_bir_lowering=False, debug=False,
                   num_devices=NCORES)

    # ---- DRAM I/O ----
    xT_d = nc.dram_tensor("xT", [F0, NTOT], f32, kind="ExternalInput").ap()
    W0_d = nc.dram_tensor("W0", [F0 * F0, H], f32, kind="ExternalInput").ap()
    W1_d = nc.dram_tensor("W1", [F0 * H, H], f32, kind="ExternalInput").ap()
    W2_d = nc.dram_tensor("W2", [F0 * H, H], f32, kind="ExternalInput").ap()
    b0_d = nc.dram_tensor("b0c", [H, 1], f32, kind="ExternalInput").ap()
    b1_d = nc.dram_tensor("b1c", [H, 1], f32, kind="ExternalInput").ap()
    b2_d = nc.dram_tensor("b2c", [H, 1], f32, kind="ExternalInput").ap()
    S_d = nc.dram_tensor("S", [F0, 8, 128], f32r, kind="ExternalInput").ap()
    E_d = nc.dram_tensor("E", [F0, F0, 128], f32r, kind="ExternalInput").ap()
    BLK_d = nc.dram_tensor("BLK", [128, TS], f32, kind="ExternalInput").ap()
    id_d = nc.dram_tensor("ident", [128, 128], f32, kind="ExternalInput").ap()
    out_d = nc.dram_tensor("out", [BL, 3 * H], f32, kind="ExternalOutput").ap()

    with tile.TileContext(nc) as tc:
        with tc.tile_pool(name="const", bufs=1) as cpool, \
             tc.tile_pool(name="zpool", bufs=4) as zpool, \
             tc.tile_pool(name="stg", bufs=1) as stg, \
             tc.tile_pool(name="h1a", bufs=2) as h1apool, \
             tc.tile_pool(name="xd", bufs=2) as xdpool, \
             tc.tile_pool(name="bcast_ps", bufs=3, space="PSUM") as bps, \
             tc.tile_pool(name="acc_ps", bufs=1, space="PSUM") as aps, \
             tc.tile_pool(name="l2_ps", bufs=3, space="PSUM") as l2ps:

            # ---- load constants / weights ----
            xT = cpool.tile([F0, NTOT], f32)
            nc.sync.dma_start(xT[:], xT_d)
            xTr = cpool.tile([F0, NTOT], f32r)
            nc.vector.tensor_copy(xTr[:], xT[:])

            W0s = stg.tile([128, F0, H], f32, tag="wstg")
            nc.sync.dma_start(W0s[:, :F0 * F0 // 128, :],
                              W0_d.rearrange("(c p) h -> p c h", p=128))
            W0r = cpool.tile([128, F0 * F0 // 128, H], f32r)
            nc.vector.tensor_copy(W0r[:], W0s[:, :F0 * F0 // 128, :])

            W1s = stg.tile([128, F0, H], f32, tag="wstg")
            nc.sync.dma_start(W1s[:], W1_d.rearrange("(c p) h -> p c h", p=128))
            W1r = cpool.tile([128, F0, H], f32r)
            nc.vector.tensor_copy(W1r[:], W1s[:])

            # W2 stays fp32 (L2 is tiny; avoids f32r pairing constraints)
            W2s = cpool.tile([128, F0, H], f32)
            nc.sync.dma_start(W2s[:], W2_d.rearrange("(c p) h -> p c h", p=128))

            b0c = cpool.tile([H, 1], f32)
            b1c = cpool.tile([H, 1], f32)
            b2c = cpool.tile([H, 1], f32)
            nc.sync.dma_start(b0c[:], b0_d)
            nc.sync.dma_start(b1c[:], b1_d)
            nc.sync.dma_start(b2c[:], b2_d)
            b2x = cpool.tile([H, 1], f32)
            nc.vector.tensor_scalar_mul(b2x[:], b2c[:], float(D))

            Sr = cpool.tile([F0, 8, 128], f32r)
            nc.sync.dma_start(Sr[:], S_d)

            BLKs = cpool.tile([128, TS], f32)
            nc.sync.dma_start(BLKs[:], BLK_d)
            ident = cpool.tile([128, 128], f32)
            nc.sync.dma_start(ident[:], id_d)

            Er = cpool.tile([F0, F0, 128], f32r)
            nc.sync.dma_start(Er[:], E_d)

            # XG[p, n] = xT[p % 32, n]
            XG = cpool.tile([128, NTOT], f32)
            for q in range(4):
                nc.vector.tensor_copy(XG[q * F0:(q + 1) * F0, :], xT[:])

            # ---- state and output tensors ----
            h0T = cpool.tile([H, NTOT], f32)
            h1T = cpool.tile([H, NTOT], f32)
            out0T = cpool.tile([H, BL], f32)
            out1T = cpool.tile([H, BL], f32)
            out2s = cpool.tile([H, BL], f32)
            XA = cpool.tile([128, NT, F0], f32)   # [(s,d), t, f]
            Pn = cpool.tile([128, NT * 128], f32)  # [g, (t, f, s)]
            out_all = cpool.tile([BL, 3 * H], f32)

            # ================= main layers, per n-chunk =================
            for j in range(NCHUNK):
                nsl = bass.ts(j, NJ)

                # ----- layer 0 -----
                h0ps = aps.tile([128, NJ], f32, tag="h0acc")
                for c in range(8):
                    x4 = bps.tile([128, NJ], f32, tag="bc")
                    with nc.named_scope("x4mm"):
                        nc.tensor.matmul(x4[:], Sr[:, c, :], xTr[:, nsl],
                                         start=True, stop=True)
                    z0 = zpool.tile([128, NJ], f32r, tag="z")
                    with nc.named_scope("z0tt"):
                        nc.vector.tensor_mul(z0[:], XG[:, nsl], x4[:])
                    with nc.named_scope("l0mm"):
                        nc.tensor.matmul(h0ps[:], W0r[:, c, :], z0[:],
                                         start=(c == 0), stop=(c == 7))
                nc.scalar.activation(h0T[:, nsl], h0ps[:],
                                     mybir.ActivationFunctionType.Identity,
                                     bias=b0c[:])

                # ----- layer 1 -----
                h1ps = aps.tile([128, NJ], f32, tag="h1acc")
                for f in range(F0):
                    xb = bps.tile([128, NJ], f32, tag="bc")
                    with nc.named_scope("xbmm"):
                        nc.tensor.matmul(xb[:], Er[:, f, :], xTr[:, nsl],
                                         start=True, stop=True)
                    z1 = zpool.tile([128, NJ], f32r, tag="z")
                    with nc.named_scope("z1tt"):
                        nc.vector.tensor_mul(z1[:], h0T[:, nsl], xb[:])
                    with nc.named_scope("l1mm"):
                        nc.tensor.matmul(h1ps[:], W1r[:, f, :], z1[:],
                                         start=(f == 0), stop=(f == F0 - 1))
                nc.scalar.activation(h1T[:, nsl], h1ps[:],
                                     mybir.ActivationFunctionType.Identity,
                                     bias=b1c[:])

                # ----- d-reductions for out0 / out1 -----
                bsl = bass.ts(j, SPJ)
                nc.vector.reduce_sum(
                    out0T[:, bsl],
                    h0T[:, nsl].rearrange("p (b d) -> p b d", d=D),
                    axis=mybir.AxisListType.X)
                nc.vector.reduce_sum(
                    out1T[:, bsl],
                    h1T[:, nsl].rearrange("p (b d) -> p b d", d=D),
                    axis=mybir.AxisListType.X)

            # ================= layer 2 (reduced bilinear form) ==========
            for t in range(NT):
              with nc.named_scope("l2"):
                tsl = bass.ts(t, 128)
                # XA_t = (xT[:, t*128:+128])^T  -> (128 rows=(s,d), 32 f)
                xa_ps = l2ps.tile([128, 128], f32, tag="l2")
                nc.tensor.transpose(xa_ps[:, :F0], xT[:, tsl], ident[:F0, :F0])
                nc.scalar.activation(XA[:, t, :], xa_ps[:, :F0],
                                     mybir.ActivationFunctionType.Copy)
                # H1A_t = (h1T[:, t*128:+128])^T -> (128 rows=(s,d), 128 g)
                h1a_ps = l2ps.tile([128, 128], f32, tag="l2")
                nc.tensor.transpose(h1a_ps[:], h1T[:, tsl], ident[:])
                h1a = h1apool.tile([128, 128], f32)
                nc.scalar.activation(h1a[:], h1a_ps[:],
                                     mybir.ActivationFunctionType.Copy)
                # XD_t[(s,d), (f,s')] = XA_t[(s,d), f] * (s == s')
                xd = xdpool.tile([128, F0, TS], f32)
                nc.vector.tensor_mul(
                    xd[:],
                    XA[:, t, :, None].to_broadcast((128, F0, TS)),
                    BLKs[:, None, :].to_broadcast((128, F0, TS)))
                # Pn_t[g, (f, s)] = sum_d h1[b_s, g, d] * x0[b_s, f, d]
                pn_ps = l2ps.tile([128, 128], f32, tag="l2")
                nc.tensor.matmul(pn_ps[:], h1a[:],
                                 xd[:].rearrange("p f s -> p (f s)"),
                                 start=True, stop=True)
                nc.scalar.activation(Pn[:, tsl], pn_ps[:],
                                     mybir.ActivationFunctionType.Copy)

            # out2T[h, b] = sum_f W2[(f,:),h]^T @ Pn[:, (:, f, :)]
            out2ps = l2ps.tile([128, BL], f32, tag="l2")
            PnV = Pn[:].rearrange("p (t f s) -> p t f s", f=F0, s=TS)
            with nc.named_scope("l2out"):
                for f in range(F0):
                    nc.tensor.matmul(out2ps[:], W2s[:, f, :], PnV[:, :, f, :],
                                     start=(f == 0), stop=(f == F0 - 1))
            nc.vector.tensor_scalar_add(out2s[:], out2ps[:], b2x[:])

            # ============ transpose outputs to (b, h) and store =========
            for k, src in enumerate((out0T, out1T, out2s)):
                ops_ = l2ps.tile([128, 128], f32, tag="l2")
                nc.tensor.transpose(ops_[:], src[:], ident[:])
                nc.scalar.activation(out_all[:, bass.ts(k, H)], ops_[:],
                                     mybir.ActivationFunctionType.Copy)
            nc.sync.dma_start(out_d, out_all[:])

    nc.compile()
    return nc


def _consts():
    S = np.zeros((F0, 8, 128), np.float32)
    for c in range(8):
        for m in range(128):
            S[4 * c + m // 32, c, m] = 1.0
    E = np.zeros((F0, F0, 128), np.float32)
    for f in range(F0):
        E[f, f, :] = 1.0
    BLK = np.zeros((128, TS), np.float32)
    for p in range(128):
        BLK[p, p // 32] = 1.0
    ident = np.eye(128, dtype=np.float32)
    return S, E, BLK, ident


def kernel(inputs, W0, W1, W2, b0, b1, b2, field_size, embedding_size):
    x0 = np.ascontiguousarray(np.asarray(inputs, np.float32).reshape(B, F0, D))
    W0 = np.ascontiguousarray(np.asarray(W0, np.float32))
    W1 = np.ascontiguousarray(np.asarray(W1, np.float32))
    W2 = np.ascontiguousarray(np.asarray(W2, np.float32))
    b0 = np.asarray(b0, np.float32).reshape(H, 1)
    b1 = np.asarray(b1, np.float32).reshape(H, 1)
    b2 = np.asarray(b2, np.float32).reshape(H, 1)

    if "nc" not in _cache:
        _cache["nc"] = _build_program()
    nc = _cache["nc"]

    S, E, BLK, ident = _consts()
    in_maps = []
    for c in range(NCORES):
        xs = x0[c * BL:(c + 1) * BL]                      # (128, 32, 32)
        xT = np.ascontiguousarray(xs.transpose(1, 0, 2).reshape(F0, NTOT))
        in_maps.append({
            "xT": xT, "W0": W0, "W1": W1, "W2": W2,
            "b0c": b0, "b1c": b1, "b2c": b2,
            "S": S.copy(), "E": E.copy(), "BLK": BLK.copy(), "ident": ident.copy(),
        })

    res = run_bass_kernel_spmd(nc, in_maps, list(range(NCORES)),
                               **_cache.get("run_kwargs", {}))
    _cache["last_result"] = res
    out = np.concatenate([res.results[c]["out"] for c in range(NCORES)], axis=0)
    return out.astype(np.float32)


# revision 8
# speedup vs baseline: 1.1329x; 1.1328x over previous
"""Trainium2 Bass kernel for CIN (Compressed Interaction Network).

Problem: B=1024, F0=32, D=32, HID=[128,128,128], linear activations.
  layer k: z_k[b,d,(f,g)] = x0[b,f,d] * s_k[b,g,d];  h_k = z_k @ W_k + b_k
  s_{k+1} = h_k;  out = concat_k sum_d h_k  -> (B, 384)

Strategy (8 cores, batch-sharded 128 samples/core), all-bf16 compute with
fp32 PSUM accumulation:
  "Orientation B" layout: features on partitions, n=(b,d) on free dim.
    xT[f, n] = x0[b,f,d]; h_kT[h, n] (matmul output layout == next layer's
    state layout: no transposes in the main chain).
  Broadcast tiles (row f of xT replicated over 128 partitions) are made by
  one-hot stationary PE matmuls -> PSUM, then ScalarE copies them to SBUF
  bf16; DVE/GpSimd build z tiles with 2x-mode bf16 multiplies; PE runs the
  GEMMs with pipelined bf16 LDWEIGHTS.
  Layer 2's full map is never materialized: out2 = vec(x0 @ h1^T) @ W2 +
  D*b2 via per-sample bilinear contractions with a block-diagonal masked
  moving operand (one 128x128 matmul per 4-sample tile).
"""
import sys

sys.path.insert(0, "/opt/trn_rl_repo")

import numpy as np
import ml_dtypes

import concourse.bass as bass
import concourse.tile as tile
from concourse import bacc, mybir
from concourse.bass_utils import run_bass_kernel_spmd

NCORES = 8
B, F0, D = 1024, 32, 32
H = 128
BL = B // NCORES          # samples per core
NTOT = BL * D             # 4096 n-columns per core
NJ = 512                  # n-chunk (one PSUM bank)
NCHUNK = NTOT // NJ       # 8
SPJ = NJ // D             # samples per n-chunk (16)
TS = 4                    # samples per 128-row tile in L2
NT = BL // TS             # 32 L2 tiles

f32 = mybir.dt.float32
bf16 = mybir.dt.bfloat16
nbf16 = ml_dtypes.bfloat16

# every GP_MOD-th z-multiply goes to GpSimd instead of DVE
GP_MOD = 4

_cache = {}


def _build_program():
    nc = bacc.Bacc("TRN2", target_bir_lowering=False, debug=False,
                   num_devices=NCORES)

    # ---- DRAM I/O (weights/constants pre-cast to bf16 on host) ----
    xT_d = nc.dram_tensor("xT", [F0, NTOT], bf16, kind="ExternalInput").ap()
    W0_d = nc.dram_tensor("W0", [F0 * F0, H], bf16, kind="ExternalInput").ap()
    W1_d = nc.dram_tensor("W1", [F0 * H, H], bf16, kind="ExternalInput").ap()
    W2_d = nc.dram_tensor("W2", [F0 * H, H], bf16, kind="ExternalInput").ap()
    b0_d = nc.dram_tensor("b0c", [H, 1], f32, kind="ExternalInput").ap()
    b1_d = nc.dram_tensor("b1c", [H, 1], f32, kind="ExternalInput").ap()
    b2_d = nc.dram_tensor("b2c", [H, 1], f32, kind="ExternalInput").ap()
    S_d = nc.dram_tensor("S", [F0, 8, 128], bf16, kind="ExternalInput").ap()
    E_d = nc.dram_tensor("E", [F0, F0, 128], bf16, kind="ExternalInput").ap()
    BLK_d = nc.dram_tensor("BLK", [128, TS], bf16, kind="ExternalInput").ap()
    idb_d = nc.dram_tensor("idb", [128, 128], bf16, kind="ExternalInput").ap()
    idf_d = nc.dram_tensor("idf", [128, 128], f32, kind="ExternalInput").ap()
    out_d = nc.dram_tensor("out", [BL, 3 * H], f32, kind="ExternalOutput").ap()

    with tile.TileContext(nc) as tc:
        with tc.tile_pool(name="const", bufs=1) as cpool, \
             tc.tile_pool(name="zpool", bufs=6) as zpool, \
             tc.tile_pool(name="xbsb", bufs=6) as xbpool, \
             tc.tile_pool(name="h1a", bufs=2) as h1apool, \
             tc.tile_pool(name="xd", bufs=2) as xdpool, \
             tc.tile_pool(name="bcast_ps", bufs=3, space="PSUM") as bps, \
             tc.tile_pool(name="acc_ps", bufs=1, space="PSUM") as aps, \
             tc.tile_pool(name="l2_ps", bufs=3, space="PSUM") as l2ps:

            # ---- load constants / weights ----
            xT = cpool.tile([F0, NTOT], bf16)
            nc.sync.dma_start(xT[:], xT_d)
            W0b = cpool.tile([128, F0 * F0 // 128, H], bf16)
            nc.sync.dma_start(W0b[:], W0_d.rearrange("(c p) h -> p c h", p=128))
            W1b = cpool.tile([128, F0, H], bf16)
            nc.sync.dma_start(W1b[:], W1_d.rearrange("(c p) h -> p c h", p=128))
            W2b = cpool.tile([128, F0, H], bf16)
            nc.sync.dma_start(W2b[:], W2_d.rearrange("(c p) h -> p c h", p=128))
            b0c = cpool.tile([H, 1], f32)
            b1c = cpool.tile([H, 1], f32)
            b2c = cpool.tile([H, 1], f32)
            nc.sync.dma_start(b0c[:], b0_d)
            nc.sync.dma_start(b1c[:], b1_d)
            nc.sync.dma_start(b2c[:], b2_d)
            b2x = cpool.tile([H, 1], f32)
            nc.vector.tensor_scalar_mul(b2x[:], b2c[:], float(D))
            Sb = cpool.tile([F0, 8, 128], bf16)
            nc.sync.dma_start(Sb[:], S_d)
            Eb = cpool.tile([F0, F0, 128], bf16)
            nc.sync.dma_start(Eb[:], E_d)
            BLKs = cpool.tile([128, TS], bf16)
            nc.sync.dma_start(BLKs[:], BLK_d)
            idb = cpool.tile([128, 128], bf16)
            nc.sync.dma_start(idb[:], idb_d)
            idf = cpool.tile([128, 128], f32)
            nc.sync.dma_start(idf[:], idf_d)

            # XG[p, n] = xT[p % 32, n]
            XG = cpool.tile([128, NTOT], bf16)
            for q in range(4):
                nc.vector.tensor_copy(XG[q * F0:(q + 1) * F0, :], xT[:])

            # ---- state and output tensors ----
            h0T = cpool.tile([H, NTOT], bf16)
            h1T = cpool.tile([H, NTOT], bf16)
            out0T = cpool.tile([H, BL], f32)
            out1T = cpool.tile([H, BL], f32)
            out2s = cpool.tile([H, BL], f32)
            XA = cpool.tile([128, NT, F0], bf16)   # [(s,d), t, f]
            Pn = cpool.tile([128, NT * 128], bf16)  # [g, (t, f, s)]
            out_all = cpool.tile([BL, 3 * H], f32)

            def zmul(i, out, a, b):
                eng = nc.gpsimd if (i % GP_MOD == GP_MOD - 1) else nc.vector
                eng.tensor_mul(out, a, b)

            # ================= main layers, per n-chunk =================
            for j in range(NCHUNK):
                nsl = bass.ts(j, NJ)

                # ----- layer 0: z0[(4f x 32g), n] = xT[f,n]*xT[g,n] -----
                h0ps = aps.tile([128, NJ], f32, tag="h0acc")
                for c in range(8):
                    x4ps = bps.tile([128, NJ], f32, tag="bc")
                    with nc.named_scope("x4mm"):
                        nc.tensor.matmul(x4ps[:], Sb[:, c, :], xT[:, nsl],
                                         start=True, stop=True)
                    x4 = xbpool.tile([128, NJ], bf16, tag="xb")
                    with nc.named_scope("x4cp"):
                        nc.scalar.activation(
                            x4[:], x4ps[:], mybir.ActivationFunctionType.Copy)
                    z0 = zpool.tile([128, NJ], bf16, tag="z")
                    with nc.named_scope("z0tt"):
                        zmul(c, z0[:], XG[:, nsl], x4[:])
                    with nc.named_scope("l0mm"):
                        nc.tensor.matmul(h0ps[:], W0b[:, c, :], z0[:],
                                         start=(c == 0), stop=(c == 7))
                with nc.named_scope("h0cp"):
                    nc.scalar.activation(h0T[:, nsl], h0ps[:],
                                         mybir.ActivationFunctionType.Identity,
                                         bias=b0c[:])

                # ----- layer 1: z1_f[g, n] = h0T[g,n]*xT[f,n] -----
                h1ps = aps.tile([128, NJ], f32, tag="h1acc")
                for f in range(F0):
                    xbps = bps.tile([128, NJ], f32, tag="bc")
                    with nc.named_scope("xbmm"):
                        nc.tensor.matmul(xbps[:], Eb[:, f, :], xT[:, nsl],
                                         start=True, stop=True)
                    xb = xbpool.tile([128, NJ], bf16, tag="xb")
                    with nc.named_scope("xbcp"):
                        nc.scalar.activation(
                            xb[:], xbps[:], mybir.ActivationFunctionType.Copy)
                    z1 = zpool.tile([128, NJ], bf16, tag="z")
                    with nc.named_scope("z1tt"):
                        zmul(f, z1[:], h0T[:, nsl], xb[:])
                    with nc.named_scope("l1mm"):
                        nc.tensor.matmul(h1ps[:], W1b[:, f, :], z1[:],
                                         start=(f == 0), stop=(f == F0 - 1))
                with nc.named_scope("h1cp"):
                    nc.scalar.activation(h1T[:, nsl], h1ps[:],
                                         mybir.ActivationFunctionType.Identity,
                                         bias=b1c[:])

                # ----- d-reductions for out0 / out1 -----
                bsl = bass.ts(j, SPJ)
                with nc.named_scope("red"):
                    nc.vector.reduce_sum(
                        out0T[:, bsl],
                        h0T[:, nsl].rearrange("p (b d) -> p b d", d=D),
                        axis=mybir.AxisListType.X)
                    nc.vector.reduce_sum(
                        out1T[:, bsl],
                        h1T[:, nsl].rearrange("p (b d) -> p b d", d=D),
                        axis=mybir.AxisListType.X)

            # ================= layer 2 (reduced bilinear form) ==========
            for t in range(NT):
              with nc.named_scope("l2"):
                tsl = bass.ts(t, 128)
                xa_ps = l2ps.tile([128, 128], bf16, tag="l2")
                nc.tensor.transpose(xa_ps[:, :F0], xT[:, tsl], idb[:F0, :F0])
                nc.scalar.activation(XA[:, t, :], xa_ps[:, :F0],
                                     mybir.ActivationFunctionType.Copy)
                h1a_ps = l2ps.tile([128, 128], bf16, tag="l2")
                nc.tensor.transpose(h1a_ps[:], h1T[:, tsl], idb[:])
                h1a = h1apool.tile([128, 128], bf16)
                nc.scalar.activation(h1a[:], h1a_ps[:],
                                     mybir.ActivationFunctionType.Copy)
                # XD_t[(s,d), (f,s')] = XA_t[(s,d), f] * (s == s')
                xd = xdpool.tile([128, F0, TS], bf16)
                nc.vector.tensor_mul(
                    xd[:],
                    XA[:, t, :, None].to_broadcast((128, F0, TS)),
                    BLKs[:, None, :].to_broadcast((128, F0, TS)))
                # Pn_t[g, (f, s)] = sum_d h1[b_s, g, d] * x0[b_s, f, d]
                pn_ps = l2ps.tile([128, 128], f32, tag="l2")
                nc.tensor.matmul(pn_ps[:], h1a[:],
                                 xd[:].rearrange("p f s -> p (f s)"),
                                 start=True, stop=True)
                nc.scalar.activation(Pn[:, tsl], pn_ps[:],
                                     mybir.ActivationFunctionType.Copy)

            out2ps = l2ps.tile([128, BL], f32, tag="l2")
            PnV = Pn[:].rearrange("p (t f s) -> p t f s", f=F0, s=TS)
            with nc.named_scope("l2out"):
                for f in range(F0):
                    nc.tensor.matmul(out2ps[:], W2b[:, f, :], PnV[:, :, f, :],
                                     start=(f == 0), stop=(f == F0 - 1))
            nc.vector.tensor_scalar_add(out2s[:], out2ps[:], b2x[:])

            # ============ transpose outputs to (b, h) and store =========
            with nc.named_scope("outtp"):
                for k, src in enumerate((out0T, out1T, out2s)):
                    ops_ = l2ps.tile([128, 128], f32, tag="l2")
                    nc.tensor.transpose(ops_[:], src[:], idf[:])
                    nc.scalar.activation(out_all[:, bass.ts(k, H)], ops_[:],
                                         mybir.ActivationFunctionType.Copy)
            nc.sync.dma_start(out_d, out_all[:])

    nc.compile()
    return nc


def _consts():
    S = np.zeros((F0, 8, 128), nbf16)
    for c in range(8):
        for m in range(128):
            S[4 * c + m // 32, c, m] = 1.0
    E = np.zeros((F0, F0, 128), nbf16)
    for f in range(F0):
        E[f, f, :] = 1.0
    BLK = np.zeros((128, TS), nbf16)
    for p in range(128):
        BLK[p, p // 32] = 1.0
    idb = np.eye(128, dtype=nbf16)
    idf = np.eye(128, dtype=np.float32)
    return S, E, BLK, idb, idf


def kernel(inputs, W0, W1, W2, b0, b1, b2, field_size, embedding_size):
    x0 = np.ascontiguousarray(np.asarray(inputs, np.float32).reshape(B, F0, D))
    W0 = np.asarray(W0, np.float32).astype(nbf16)
    W1 = np.asarray(W1, np.float32).astype(nbf16)
    W2 = np.asarray(W2, np.float32).astype(nbf16)
    b0 = np.asarray(b0, np.float32).reshape(H, 1)
    b1 = np.asarray(b1, np.float32).reshape(H, 1)
    b2 = np.asarray(b2, np.float32).reshape(H, 1)

    if "nc" not in _cache:
        _cache["nc"] = _build_program()
    nc = _cache["nc"]

    S, E, BLK, idb, idf = _consts()
    in_maps = []
    for c in range(NCORES):
        xs = x0[c * BL:(c + 1) * BL]                      # (128, 32, 32)
        xT = np.ascontiguousarray(
            xs.transpose(1, 0, 2).reshape(F0, NTOT)).astype(nbf16)
        in_maps.append({
            "xT": xT, "W0": W0, "W1": W1, "W2": W2,
            "b0c": b0, "b1c": b1, "b2c": b2,
            "S": S.copy(), "E": E.copy(), "BLK": BLK.copy(),
            "idb": idb.copy(), "idf": idf.copy(),
        })

    res = run_bass_kernel_spmd(nc, in_maps, list(range(NCORES)),
                               **_cache.get("run_kwargs", {}))
    _cache["last_result"] = res
    out = np.concatenate([res.results[c]["out"] for c in range(NCORES)], axis=0)
    return out.astype(np.float32)
